# revision 1
# baseline (speedup 1.0000x reference)
"""Trainium2 Bass kernel for nn_Attention_Block (dense transformer block).

Strategy: pure data-parallel over batch — 8 samples, 8 NeuronCores, one
sample per core, weights replicated, no collectives. Per core everything
stays channels-on-partitions (c x n layout):

  GN1 (bn_stats + selector-matmul group reduce) -> QKV matmul (bf16) ->
  per-head attention (scores computed pre-transposed K^T Q, softmax via
  exp + ones-column denominator in the AV matmul) -> out-proj -> GN2 ->
  SwiGLU MLP (sigmoid+mults) -> +residual.

Matmuls run in bf16 (full PE rate); stats/softmax denominators in fp32.
"""

import os

import numpy as np
import ml_dtypes

KSTAGE = int(os.environ.get("KSTAGE", "7"))

C = 512
NSP = 1024  # 32*32 spatial
CT = 4  # channel tiles of 128
HEADS = 8
D = 64
HID = 2048
EPS = 1e-5

_cache = {}


def _patch_tile_drain(tile, mybir):
    """walrus in this environment accepts very few sync waits per
    instruction; the TileContext tail drain carries one wait per proc of
    the global clock. Split them across preceding SP drains."""
    if getattr(tile.TileContext, "_drain_patched", False):
        return

    def _patched(self, tick_clock, wait_clock):
        nc = self.nc
        spills = [nc.sync.drain() for _ in range(40)]
        drain_inst = nc.sync.drain()
        wait_clock.add_sem_waits(
            drain_inst.ins, tile.ScopedClock({None: tick_clock.global_clock})
        )
        si = drain_inst.ins.sync_info
        waits = list(si.on_wait) if si is not None and si.on_wait else []
        upds = list(si.on_update) if si is not None and si.on_update else []
        if len(waits) > 1:
            *pre, last = waits
            assert len(pre) <= len(spills), "too many drain wait chunks"
            for sp_inst, w in zip(spills, pre):
                sp_inst.ins.sync_info = mybir.SyncInfo(on_wait=[w], on_update=[])
            drain_inst.ins.sync_info = mybir.SyncInfo(on_wait=[last], on_update=upds)
        nc.all_engine_barrier()
        assert self.sems is not None
        popped = nc._tile_sem_poison_stack.pop()
        assert popped is self._sem_poison
        nc.clear_and_free_semaphores(list(self.sems.allocated().values()))
        nc.all_engine_barrier()

    tile.TileContext._drain_and_barrier = _patched
    tile.TileContext._drain_patched = True


def _split_multi_waits(nc, mybir, maxw=1):
    """Hoist extra sync waits onto same-engine EventSemaphore carriers so
    no instruction carries more than `maxw` waits."""
    f = nc.m.functions[0]
    for bb in f.blocks:
        insts = list(bb.instructions)
        need = [
            i
            for i in insts
            if getattr(i, "sync_info", None)
            and i.sync_info.on_wait
            and len(i.sync_info.on_wait) > maxw
        ]
        if not need:
            continue
        carriers = {}
        for inst in need:
            w = list(inst.sync_info.on_wait)
            upds = list(inst.sync_info.on_update) if inst.sync_info.on_update else []
            keep = w[-maxw:]
            extra = w[:-maxw]
            cs = []
            for i in range(0, len(extra), maxw):
                c = mybir.InstEventSemaphore(
                    name=f"I-waitc-{nc.next_id()}", ins=[], outs=[]
                )
                c.engine = inst.engine
                c.sync_info = mybir.SyncInfo(on_wait=extra[i : i + maxw], on_update=[])
                nc.register_instruction(c)
                cs.append(c)
            inst.sync_info = mybir.SyncInfo(on_wait=keep, on_update=upds)
            carriers[inst.name] = cs
        carrier_names = {c.name for cs in carriers.values() for c in cs}
        rebuilt = []
        for inst in list(bb.instructions):
            if inst.name in carrier_names:
                continue
            if inst.name in carriers:
                rebuilt.extend(carriers[inst.name])
            rebuilt.append(inst)
        bb.instructions = rebuilt


def _build_nc():
    import concourse.bass as bass
    import concourse.tile as tile
    from concourse import mybir

    _patch_tile_drain(tile, mybir)

    F32 = mybir.dt.float32
    BF16 = mybir.dt.bfloat16
    ADD = mybir.AluOpType.add
    SUB = mybir.AluOpType.subtract
    MULT = mybir.AluOpType.mult
    AF = mybir.ActivationFunctionType

    nc = bass.Bass()

    x_d = nc.declare_dram_parameter("x", [C, NSP], F32, isOutput=False)
    wqkv_d = nc.declare_dram_parameter("wqkvT", [C, 3 * C], BF16, isOutput=False)
    qkvb_d = nc.declare_dram_parameter("qkvb", [128, 12], F32, isOutput=False)
    wo_d = nc.declare_dram_parameter("woT", [C, C], BF16, isOutput=False)
    outb_d = nc.declare_dram_parameter("outb", [128, 4], F32, isOutput=False)
    g1_d = nc.declare_dram_parameter("g1", [128, 4], F32, isOutput=False)
    b1_d = nc.declare_dram_parameter("b1", [128, 4], F32, isOutput=False)
    g2_d = nc.declare_dram_parameter("g2", [128, 4], F32, isOutput=False)
    b2_d = nc.declare_dram_parameter("b2", [128, 4], F32, isOutput=False)
    w1_d = nc.declare_dram_parameter("w1T", [C, 2 * HID], BF16, isOutput=False)
    w2_d = nc.declare_dram_parameter("w2T", [HID, C], BF16, isOutput=False)
    sel_d = nc.declare_dram_parameter("sel", [C, 32], F32, isOutput=False)
    selT_d = nc.declare_dram_parameter("selT", [32, C], F32, isOutput=False)
    id_d = nc.declare_dram_parameter("ident", [128, 128], BF16, isOutput=False)
    selbc_d = nc.declare_dram_parameter("selbc", [16, 1024], BF16, isOutput=False)
    out_d = nc.declare_dram_parameter("out", [C, NSP], F32, isOutput=True)

    with tile.TileContext(nc) as tc:
        with (
            tc.tile_pool(name="pers", bufs=1) as pers,
            tc.tile_pool(name="gnp", bufs=2) as gnp,
            tc.tile_pool(name="expp", bufs=16) as expp,
            tc.tile_pool(name="vtp", bufs=2) as vtp,
            tc.tile_pool(name="swp", bufs=2) as swp,
            tc.tile_pool(name="unp", bufs=2) as unp,
            tc.tile_pool(name="kpp", bufs=2) as kpp,
            tc.tile_pool(name="invp", bufs=2) as invp,
            tc.tile_pool(name="ps", bufs=4, space="PSUM") as ps_pool,
        ):
            def pstile(shape, dtype):
                return ps_pool.tile(shape, dtype, tag="ps", name="ps")

            # ---- input loads (x lands in the attn2 slots; reloaded later) ----
            x_sb = []
            for t in range(CT):
                xt = pers.tile([128, NSP], F32, tag=f"attn2{t}", name=f"attn2{t}")
                nc.sync.dma_start(xt[:], x_d[t * 128 : (t + 1) * 128, :])
                x_sb.append(xt)
            sel_sb = []
            for t in range(CT):
                st = pers.tile([128, 32], F32, tag=f"sel{t}", name=f"sel{t}")
                nc.sync.dma_start(st[:], sel_d[t * 128 : (t + 1) * 128, :])
                sel_sb.append(st)
            selT_sb = pers.tile([32, C], F32, tag="selT", name="selT")
            nc.sync.dma_start(selT_sb[:], selT_d[:, :])
            g1_sb = pers.tile([128, 4], F32, tag="g1", name="g1")
            nc.sync.dma_start(g1_sb[:], g1_d[:, :])
            b1_sb = pers.tile([128, 4], F32, tag="b1", name="b1")
            nc.sync.dma_start(b1_sb[:], b1_d[:, :])
            wqkv_sb = []
            for k in range(CT):
                wt = pers.tile([128, 3 * C], BF16, tag=f"wqkv{k}", name=f"wqkv{k}")
                nc.sync.dma_start(wt[:], wqkv_d[k * 128 : (k + 1) * 128, :])
                wqkv_sb.append(wt)
            qkvb_sb = pers.tile([128, 12], F32, tag="qkvb", name="qkvb")
            nc.sync.dma_start(qkvb_sb[:], qkvb_d[:, :])
            selbc_sb = pers.tile([16, 1024], BF16, tag="selbc", name="selbc")
            nc.sync.dma_start(selbc_sb[:], selbc_d[:, :])
            id_sb = pers.tile([128, 128], BF16, tag="ident", name="ident")
            nc.sync.dma_start(id_sb[:], id_d[:, :])
            wo_sb = []
            for k in range(CT):
                wt = pers.tile([128, C], BF16, tag=f"wo{k}", name=f"wo{k}")
                nc.sync.dma_start(wt[:], wo_d[k * 128 : (k + 1) * 128, :])
                wo_sb.append(wt)
            outb_sb = pers.tile([128, 4], F32, tag="outb", name="outb")
            nc.sync.dma_start(outb_sb[:], outb_d[:, :])
            g2_sb = pers.tile([128, 4], F32, tag="g2", name="g2")
            nc.sync.dma_start(g2_sb[:], g2_d[:, :])
            b2_sb = pers.tile([128, 4], F32, tag="b2", name="b2")
            nc.sync.dma_start(b2_sb[:], b2_d[:, :])
            w1_sb = []
            for k in range(CT):
                wt = pers.tile([128, 2 * HID], BF16, tag=f"w1{k}", name=f"w1{k}")
                nc.sync.dma_start(wt[:], w1_d[k * 128 : (k + 1) * 128, :])
                w1_sb.append(wt)
            w2_sb = []
            for k in range(16):
                wt = pers.tile([128, C], BF16, tag=f"w2{k}", name=f"w2{k}")
                nc.sync.dma_start(wt[:], w2_d[k * 128 : (k + 1) * 128, :])
                w2_sb.append(wt)

            eps32 = pers.tile([32, 1], F32, tag="eps", name="eps")
            nc.vector.memset(eps32[:], EPS)
            ones65 = pers.tile([65, 1], F32, tag="ones65", name="ones65")
            nc.vector.memset(ones65[:], 1.0)

            # ---- group norm helper (32 groups of 16 channels x 1024) ----
            def group_norm(src_tiles, gam_sb, bet_sb, dst_tiles):
                rhs3 = []
                for t in range(CT):
                    stats = gnp.tile([128, 2, 6], F32, tag="gn_stats", name="gn_stats")
                    for j2 in range(2):
                        nc.vector.bn_stats(
                            stats[:, j2, :], src_tiles[t][:, j2 * 512 : (j2 + 1) * 512]
                        )
                    mv = gnp.tile([128, 2], F32, tag="gn_mv", name="gn_mv")
                    nc.vector.bn_aggr(mv[:], stats[:])
                    r3 = gnp.tile([128, 3], F32, tag=f"gn_r3_{t}", name=f"gn_r3_{t}")
                    nc.vector.tensor_copy(r3[:, 0:2], mv[:])
                    nc.vector.tensor_mul(r3[:, 2:3], mv[:, 0:1], mv[:, 0:1])
                    rhs3.append(r3)
                pg = pstile([32, 3], F32)
                for t in range(CT):
                    nc.tensor.matmul(
                        pg[:], sel_sb[t][:], rhs3[t][:], start=(t == 0), stop=(t == 3)
                    )
                gs = gnp.tile([32, 2], F32, tag="gn_gs", name="gn_gs")
                tmp = gnp.tile([32, 2], F32, tag="gn_tmp", name="gn_tmp")
                pgs = gnp.tile([32, 3], F32, tag="gn_pgs", name="gn_pgs")
                nc.vector.tensor_copy(pgs[:], pg[:])
                # mean_g, E[x^2]_g, var_g, rstd_g
                nc.vector.tensor_scalar_mul(gs[:, 0:1], pgs[:, 0:1], 1.0 / 16)
                nc.vector.tensor_tensor(tmp[:, 0:1], pgs[:, 1:2], pgs[:, 2:3], op=ADD)
                nc.vector.tensor_scalar_mul(tmp[:, 0:1], tmp[:, 0:1], 1.0 / 16)
                nc.vector.tensor_mul(tmp[:, 1:2], gs[:, 0:1], gs[:, 0:1])
                nc.vector.tensor_tensor(tmp[:, 0:1], tmp[:, 0:1], tmp[:, 1:2], op=SUB)
                nc.scalar.activation(
                    tmp[:, 0:1], tmp[:, 0:1], AF.Sqrt, bias=eps32[:]
                )
                nc.vector.reciprocal(gs[:, 1:2], tmp[:, 0:1])
                for t in range(CT):
                    pbc = pstile([128, 2], F32)
                    nc.tensor.matmul(
                        pbc[:],
                        selT_sb[:, t * 128 : (t + 1) * 128],
                        gs[:],
                        start=True,
                        stop=True,
                    )
                    a_t = gnp.tile([128, 1], F32, tag="gn_A", name="gn_A")
                    b_t = gnp.tile([128, 1], F32, tag="gn_B", name="gn_B")
                    nc.vector.tensor_mul(a_t[:], pbc[:, 1:2], gam_sb[:, t : t + 1])
                    nc.vector.tensor_mul(b_t[:], pbc[:, 0:1], a_t[:])
                    nc.vector.tensor_tensor(
                        b_t[:], bet_sb[:, t : t + 1], b_t[:], op=SUB
                    )
                    nc.vector.tensor_scalar(
                        dst_tiles[t][:],
                        src_tiles[t][:],
                        scalar1=a_t[:],
                        scalar2=b_t[:],
                        op0=MULT,
                        op1=ADD,
                    )

            # ---- GN1 -> xn (bf16) ----
            xn = [pers.tile([128, NSP], BF16, tag=f"xn{t}", name=f"xn{t}") for t in range(CT)]
            group_norm(x_sb, g1_sb, b1_sb, xn)

            def dump_and_finish(tiles, cast=True, reuse=None):
                for t in range(CT):
                    if cast:
                        if reuse is not None:
                            ft = reuse[t]
                        else:
                            ft = pers.tile(
                                [128, NSP], F32, tag=f"dump{t}", name=f"dump{t}"
                            )
                        nc.vector.tensor_copy(ft[:], tiles[t][:])
                    else:
                        ft = tiles[t]
                    nc.sync.dma_start(out_d[t * 128 : (t + 1) * 128, :], ft[:])

            if KSTAGE == 1:
                dump_and_finish(xn)
                return nc, tc

            # ---- QKV (12 out tiles of 128 x 1024, bf16, bias added) ----
            qkv = [pers.tile([128, NSP], BF16, tag=f"qkv{m}", name=f"qkv{m}") for m in range(12)]
            for m in range(12):
                ps = pstile([128, NSP], F32)
                for n2 in range(2):
                    s = slice(n2 * 512, (n2 + 1) * 512)
                    for k in range(CT):
                        nc.tensor.matmul(
                            ps[:, s],
                            wqkv_sb[k][:, m * 128 : (m + 1) * 128],
                            xn[k][:, s],
                            start=(k == 0),
                            stop=(k == 3),
                        )
                nc.vector.tensor_scalar_add(qkv[m][:], ps[:], qkvb_sb[:, m : m + 1])

            if KSTAGE == 2:
                dump_and_finish(qkv[0:4])
                return nc, tc

            # ---- attention (head pairs: even rows 0:64, odd 64:128; QK pair
            # matmuls adjacent so PE runs them on disjoint row groups) ----
            xattn = [
                pers.tile([128, NSP], BF16, tag=f"xattn{t}", name=f"xattn{t}")
                for t in range(CT)
            ]
            vts = []
            for _vi in range(2):
                _vt = vtp.tile([128, 8, 224], BF16, tag="vt", name="vt")
                nc.vector.memset(_vt[:], 0.0)
                nc.vector.memset(_vt[:, :, 64:65], 1.0)
                nc.vector.memset(_vt[:, :, 130:131], 1.0)
                vts.append(_vt)
            kpe = kpp.tile([128, NSP], BF16, tag="kpe", name="kpe")
            kpo = kpp.tile([128, NSP], BF16, tag="kpo", name="kpo")
            nc.vector.memset(kpe[64:128, :], 0.0)
            nc.vector.memset(kpo[0:64, :], 0.0)
            kp = [kpe, kpo]
            for j in range(4):
                vt = vts[j % 2]
                for mk in range(8):
                    pv = pstile([128, 128], BF16)
                    nc.tensor.transpose(
                        pv[:], qkv[8 + j][:, mk * 128 : (mk + 1) * 128], id_sb[:]
                    )
                    nc.vector.tensor_copy(vt[:, mk, 0:64], pv[:, 0:64])
                    nc.vector.tensor_copy(vt[:, mk, 66:130], pv[:, 64:128])
                # zero-padded K (zero halves persist; only data halves
                # refreshed each pair, off the DVE critical path)
                nc.gpsimd.tensor_copy(kpe[0:64, :], qkv[4 + j][0:64, :])
                nc.gpsimd.tensor_copy(kpo[64:128, :], qkv[4 + j][64:128, :])
                exps = [[], []]
                for mk in range(8):
                    pqks = [pstile([128, NSP], F32), pstile([128, NSP], F32)]
                    for n2 in range(2):
                        s = slice(n2 * 512, (n2 + 1) * 512)
                        for side in range(2):
                            nc.tensor.matmul(
                                pqks[side][:, s],
                                kp[side][:, mk * 128 : (mk + 1) * 128],
                                qkv[j][:, s],
                                start=True,
                                stop=True,
                            )
                    for side in range(2):
                        e = expp.tile([128, NSP], BF16, tag="exp", name="exp")
                        nc.scalar.activation(e[:], pqks[side][:], AF.Exp, scale=0.125)
                        exps[side].append(e)
                uns = []
                for side in range(2):
                    off = 66 * side
                    pav = pstile([128, NSP], F32)
                    for n2 in range(2):
                        s = slice(n2 * 512, (n2 + 1) * 512)
                        for mk in range(8):
                            nc.tensor.matmul(
                                pav[:, s],
                                vt[:, mk, off : off + 128],
                                exps[side][mk][:, s],
                                start=(mk == 0),
                                stop=(mk == 7),
                            )
                    un = unp.tile([65, NSP], F32, tag="un", name="un")
                    nc.scalar.copy(un[:], pav[0:65, :])
                    uns.append(un)
                # denominators: PE-transpose the two denom rows into
                # partitions, one cheap parallel reciprocal, DMA back in
                # nq-order, broadcast-read, normalize.
                pdt = pstile([128, 16], F32)
                for side in range(2):
                    for jj in range(8):
                        nc.tensor.transpose(
                            pdt[:, side * 8 + jj : side * 8 + jj + 1],
                            uns[side][64:65, jj * 128 : (jj + 1) * 128],
                            ones65[64:65, 0:1],
                        )
                inv16 = invp.tile([128, 16], F32, tag="inv", name="inv")
                nc.vector.reciprocal(inv16[:], pdt[:])
                inv16b = invp.tile([128, 16], BF16, tag="invb16", name="invb16")
                nc.vector.tensor_copy(inv16b[:], inv16[:])
                # one transpose puts the reciprocals row-major; selector
                # matmuls then fan each 128-chunk across 64 partitions
                ptv = pstile([16, 128], BF16)
                nc.tensor.transpose(ptv[:], inv16b[:], id_sb[:])
                pts = invp.tile([16, 128], BF16, tag="pts", name="pts")
                nc.vector.tensor_copy(pts[:], ptv[:])
                for side in range(2):
                    pinvb = pstile([64, NSP], F32)
                    for jj in range(8):
                        r = side * 8 + jj
                        nc.tensor.matmul(
                            pinvb[:, jj * 128 : (jj + 1) * 128],
                            selbc_sb[:, r * 64 : (r + 1) * 64],
                            pts[:],
                            start=True,
                            stop=True,
                        )
                    nc.vector.tensor_mul(
                        xattn[j][64 * side : 64 * side + 64, :],
                        uns[side][0:64, :],
                        pinvb[0:64, :],
                    )

            if KSTAGE == 3:
                dump_and_finish(xattn)
                return nc, tc

            # ---- out projection (keep f32 for GN2 stats) ----
            attn2 = [
                pers.tile([128, NSP], F32, tag=f"attn2{t}", name=f"attn2{t}")
                for t in range(CT)
            ]
            for m in range(CT):
                ps = pstile([128, NSP], F32)
                for n2 in range(2):
                    s = slice(n2 * 512, (n2 + 1) * 512)
                    for k in range(CT):
                        nc.tensor.matmul(
                            ps[:, s],
                            wo_sb[k][:, m * 128 : (m + 1) * 128],
                            xattn[k][:, s],
                            start=(k == 0),
                            stop=(k == 3),
                        )
                nc.vector.tensor_scalar_add(attn2[m][:], ps[:], outb_sb[:, m : m + 1])

            if KSTAGE == 4:
                dump_and_finish(attn2, cast=False)
                return nc, tc

            # ---- GN2 -> xg (reuse xn tiles) ----
            group_norm(attn2, g2_sb, b2_sb, xn)

            if KSTAGE == 5:
                dump_and_finish(xn, reuse=attn2)
                return nc, tc

            # ---- MLP1 + SwiGLU -> act tiles (16 x (128, 1024) bf16) ----
            act = [qkv[i] for i in range(12)] + [
                expp.tile([128, NSP], BF16, tag="exp", name="exp") for i in range(4)
            ]
            for mp in range(16):
                ps1 = pstile([128, NSP], F32)
                for n2 in range(2):
                    s = slice(n2 * 512, (n2 + 1) * 512)
                    for k in range(CT):
                        nc.tensor.matmul(
                            ps1[:, s],
                            w1_sb[k][:, mp * 128 : (mp + 1) * 128],
                            xn[k][:, s],
                            start=(k == 0),
                            stop=(k == 3),
                        )
                ps2 = pstile([128, NSP], F32)
                for n2 in range(2):
                    s = slice(n2 * 512, (n2 + 1) * 512)
                    for k in range(CT):
                        nc.tensor.matmul(
                            ps2[:, s],
                            w1_sb[k][:, (mp + 16) * 128 : (mp + 17) * 128],
                            xn[k][:, s],
                            start=(k == 0),
                            stop=(k == 3),
                        )
                sg = swp.tile([128, NSP], BF16, tag="sw", name="sw")
                nc.scalar.activation(sg[:], ps1[:], AF.Silu)
                nc.vector.tensor_mul(act[mp][:], sg[:], ps2[:])

            if KSTAGE == 6:
                dump_and_finish(act[0:4], reuse=attn2)
                return nc, tc

            # reload x into the attn2 slots (attention result consumed by GN2)
            for t in range(CT):
                nc.sync.dma_start(attn2[t][:], x_d[t * 128 : (t + 1) * 128, :])

            # ---- MLP2 + residual -> out ----
            for m in range(CT):
                ps = pstile([128, NSP], F32)
                for n2 in range(2):
                    s = slice(n2 * 512, (n2 + 1) * 512)
                    for k in range(16):
                        nc.tensor.matmul(
                            ps[:, s],
                            w2_sb[k][:, m * 128 : (m + 1) * 128],
                            act[k][:, s],
                            start=(k == 0),
                            stop=(k == 15),
                        )
                nc.vector.tensor_tensor(attn2[m][:], ps[:], attn2[m][:], op=ADD)
                nc.sync.dma_start(out_d[m * 128 : (m + 1) * 128, :], attn2[m][:])

    return nc


def _get_nc():
    key = ("nc", KSTAGE)
    if key not in _cache:
        import concourse.bass  # noqa: F401  ensure importable before build
        from concourse import mybir

        res = _build_nc()
        nc = res[0] if isinstance(res, tuple) else res
        _split_multi_waits(nc, mybir, maxw=1)
        _cache[key] = nc
    return _cache[key]


def _prep_weights(inputs):
    bf = ml_dtypes.bfloat16
    f32 = np.float32

    def col4(v):  # (512,) -> (128, 4) with [p, t] = v[128t + p]
        return np.ascontiguousarray(v.reshape(4, 128).T.astype(f32))

    qkv_b = inputs["qkv_b"].astype(f32)
    sel = np.zeros((C, 32), f32)
    sel[np.arange(C), np.arange(C) // 16] = 1.0
    selbc = np.zeros((16, 1024), f32)
    for r in range(16):
        selbc[r, r * 64 : (r + 1) * 64] = 1.0
    selbc = selbc.astype(bf)

    shared = {
        "wqkvT": np.ascontiguousarray(inputs["qkv_w"].astype(f32).T).astype(bf),
        "qkvb": np.ascontiguousarray(qkv_b.reshape(12, 128).T.astype(f32)),
        "woT": np.ascontiguousarray(inputs["out_w"].astype(f32).T).astype(bf),
        "outb": col4(inputs["out_b"].astype(f32)),
        "g1": col4(inputs["gn1_gamma"].astype(f32)),
        "b1": col4(inputs["gn1_beta"].astype(f32)),
        "g2": col4(inputs["gn2_gamma"].astype(f32)),
        "b2": col4(inputs["gn2_beta"].astype(f32)),
        "w1T": np.ascontiguousarray(inputs["mlp1_w"].astype(f32).T).astype(bf),
        "w2T": np.ascontiguousarray(inputs["mlp2_w"].astype(f32).T).astype(bf),
        "sel": sel,
        "selT": np.ascontiguousarray(sel.T),
        "ident": np.eye(128, dtype=f32).astype(bf),
        "selbc": selbc,
    }
    return shared


def kernel(**inputs):
    from concourse.bass_utils import run_bass_kernel_spmd

    nc = _get_nc()
    shared = _prep_weights(inputs)
    x = np.asarray(inputs["x"], dtype=np.float32).reshape(8, C, NSP)
    in_maps = [dict(shared, x=np.ascontiguousarray(x[i])) for i in range(8)]
    res = run_bass_kernel_spmd(nc, in_maps, core_ids=list(range(8))).results
    out = np.stack([res[i]["out"] for i in range(8)], axis=0)
    return out.reshape(8, C, 32, 32).astype(np.float32)

